# revision 1
# baseline (speedup 1.0000x reference)
"""Trainium2 Bass kernel for ComputeGsct.

Math (per batch b, reduced over N voxels):
    kai(n)   = 10*x2[n,0] - i * x2[n,1]/(OMEGA*EPS0)          (complex scalar)
    A_n      = kai(n) * Gsr_n                                  (complex 3x3)
    C_b      = sum_n A_n @ Grf_n                               (complex 3x3)
    out[b,m,:] = (Re C_b, Im C_b) flattened row-major.

Strategy:
  - Batch-parallel sharding: 8 cores x 4 batches each, full N per core.
    Output is concatenated on host - no cross-core reduction needed.
  - Per 128-voxel chunk, ONE fp32 TensorE matmul with stationary
    A[128,18] (interleaved re/im, component-major) and moving
    Grf[128,18], accumulated into a PSUM [18,18] tile across all of N.
    The [18,18] result contains all four real-product combinations
    (a_r*g_r, a_r*g_i, a_i*g_r, a_i*g_i) summed over voxels; a tiny
    host-side index fixup extracts the 27 complex entries of C_b.
  - kai-scaling of Gsr runs on VectorE (two broadcast multiplies) and
    GPSIMD (the two combine ops), overlapped with DMA and TensorE.

HW-time budget per core (target_regime=memory): ~80 MB of input at
~358 GB/s => ~222 us DMA floor; all compute engines fit below that.
"""

import sys

import numpy as np

_TRN_REPO = "/opt/trn_rl_repo"
if _TRN_REPO not in sys.path:
    sys.path.insert(0, _TRN_REPO)

_PAI = 3.141592653589793
_C = 299792458.0
_OMEGA = 2.0 * _PAI * 2.4e9
_MU0 = 4.0 * _PAI * 1e-7
_EPSILON0 = 1.0 / (_C**2 * _MU0)
_KI_SCALE = -1.0 / (_OMEGA * _EPSILON0)

B_FULL, N_FULL = 32, 131072
N_CORES = 8
B_PC = B_FULL // N_CORES  # batches per core
P = 128  # SBUF partitions == matmul contraction size
KGRP = 4  # voxel-chunks fused per matmul (diag-block trick)


def build_nc(b_pc=B_PC, n=N_FULL, q=128, repeat=1, mode="full"):
    """Build the per-core Bass program (SPMD: same program, per-core data).

    repeat>1 wraps the whole computation in a device-side For_i loop; used
    only for benchmarking (wall-time slope over repeat = pure HW time).
    mode: "full" | "dma" (loads only) | "nope" (no matmuls) | "novec"
    (no kai scaling; matmuls on raw data) — HW ablation experiments.
    """
    from contextlib import ExitStack

    import concourse.bacc as bacc
    import concourse.mybir as mybir
    from concourse import tile
    from concourse.bass import ts

    f32 = mybir.dt.float32
    f16 = mybir.dt.float16
    FD = 18 * KGRP
    nc = bacc.Bacc("TRN2", target_bir_lowering=False, debug=False)

    x0 = nc.dram_tensor("x0", [b_pc, n, 9, 2], f32, kind="ExternalInput")
    x1 = nc.dram_tensor("x1", [b_pc, n, 9, 2], f32, kind="ExternalInput")
    x2 = nc.dram_tensor("x2", [b_pc, n, 2], f32, kind="ExternalInput")
    out = nc.dram_tensor("out", [FD, b_pc * FD], f32, kind="ExternalOutput")

    tile_v = P * q  # voxels per tile iteration
    assert n % tile_v == 0 and q % KGRP == 0
    n_tiles = n // tile_v

    with ExitStack() as ctx:
        tc = ctx.enter_context(tile.TileContext(nc))
        io = ctx.enter_context(tc.tile_pool(name="io", bufs=4))
        work = ctx.enter_context(tc.tile_pool(name="work", bufs=2))
        psum = ctx.enter_context(tc.tile_pool(name="psum", bufs=2, space="PSUM"))
        outp = ctx.enter_context(tc.tile_pool(name="outp", bufs=1))

        if repeat > 1:
            loop = ctx.enter_context(tc.For_i(0, repeat, 1))  # noqa: F841

        stage = outp.tile([FD, b_pc * FD], f32)

        for b in range(b_pc):
            ps = psum.tile([FD, FD], f32, tag="ps")
            for t in range(n_tiles):
                # ---- loads: voxel v = t*tile_v + p*q + qq, contiguous per partition
                g0 = io.tile([P, q * 18], f32, tag="g0")
                nc.sync.dma_start(
                    g0[:],
                    x0[b, ts(t, tile_v)].rearrange("(p qq) m r -> p (qq m r)", p=P),
                )
                # g1 goes on the ACT HWDGE ring so the two big loads stream on
                # independent rings (completion latency doesn't serialize).
                g1 = io.tile([P, q * 18], f32, tag="g1")
                nc.scalar.dma_start(
                    g1[:],
                    x1[b, ts(t, tile_v)].rearrange("(p qq) m r -> p (qq m r)", p=P),
                )
                xk = io.tile([P, q * 2], f32, tag="xk")
                nc.sync.dma_start(
                    xk[:],
                    x2[b, ts(t, tile_v)].rearrange("(p qq) r -> p (qq r)", p=P),
                )

                if mode == "dma":
                    # consume the loads so DCE keeps them
                    nc.scalar.copy(stage[0:1, 0:18], g0[0:1, 0:18])
                    nc.scalar.copy(stage[1:2, 0:18], g1[0:1, 0:18])
                    nc.scalar.copy(stage[2:3, 0:2], xk[0:1, 0:2])
                    continue
                if mode == "novec":
                    g0h = work.tile([P, q * 18], f16, tag="g0h")
                    nc.scalar.copy(g0h[:], g0[:])
                    g1h = work.tile([P, q * 18], f16, tag="g1h")
                    nc.vector.tensor_copy(g1h[:], g1[:])
                    nc.scalar.copy(stage[2:3, 0:2], xk[0:1, 0:2])
                    g0hv = g0h[:].rearrange("p (g c) -> p g c", c=18 * KGRP)
                    g1hv = g1h[:].rearrange("p (g c) -> p g c", c=18 * KGRP)
                    n_grp = q // KGRP
                    for g in range(n_grp):
                        nc.tensor.matmul(
                            ps[:],
                            g0hv[:, g, :],
                            g1hv[:, g, :],
                            start=(t == 0 and g == 0),
                            stop=(t == n_tiles - 1 and g == n_grp - 1),
                        )
                    continue

                # ---- kai components (ScalarE)
                xkv = xk[:].rearrange("p (qq r) -> p qq r", r=2)
                kr = work.tile([P, q], f32, tag="kr")
                nc.scalar.mul(kr[:], xkv[:, :, 0], 10.0)
                ki = work.tile([P, q], f32, tag="ki")
                nc.scalar.mul(ki[:], xkv[:, :, 1], _KI_SCALE)

                # ---- A = kai * Gsr (complex), interleaved re/im layout
                # tt = g0 * kr (GPSIMD), ww = g0 * ki (DVE): broadcast over the
                # 18 components of each voxel.
                g0v = g0[:].rearrange("p (qq c) -> p qq c", c=18)
                tt = work.tile([P, q * 18], f32, tag="tt")
                nc.gpsimd.tensor_mul(
                    tt[:].rearrange("p (qq c) -> p qq c", c=18),
                    g0v,
                    kr[:].unsqueeze(2).broadcast_to((P, q, 18)),
                )
                ww = work.tile([P, q * 18], f32, tag="ww")
                nc.vector.tensor_mul(
                    ww[:].rearrange("p (qq c) -> p qq c", c=18),
                    g0v,
                    ki[:].unsqueeze(2).broadcast_to((P, q, 18)),
                )
                # a_re = t_re - w_im ; a_im = t_im + w_re   (fp16 out for PE)
                aa = work.tile([P, q * 18], f16, tag="aa")
                a4 = aa[:].rearrange("p (qq m r) -> p qq m r", m=9, r=2)
                t4 = tt[:].rearrange("p (qq m r) -> p qq m r", m=9, r=2)
                w4 = ww[:].rearrange("p (qq m r) -> p qq m r", m=9, r=2)
                nc.vector.tensor_sub(a4[:, :, :, 0], t4[:, :, :, 0], w4[:, :, :, 1])
                nc.vector.tensor_add(a4[:, :, :, 1], t4[:, :, :, 1], w4[:, :, :, 0])

                # ---- moving operand in fp16 (ScalarE convert)
                g1h = work.tile([P, q * 18], f16, tag="g1h")
                nc.scalar.copy(g1h[:], g1[:])

                if mode == "nope":
                    nc.scalar.copy(stage[0:1, 0:18], aa[0:1, 0:18])
                    nc.scalar.copy(stage[1:2, 0:18], g1h[0:1, 0:18])
                    continue

                # ---- TensorE: per K-chunk group, one [128,72]^T@[128,72]
                # matmul; the 4 diagonal [18,18] blocks accumulate the wanted
                # per-chunk products (off-diagonal blocks are ignored).
                av = aa[:].rearrange("p (g c) -> p g c", c=18 * KGRP)
                g1hv = g1h[:].rearrange("p (g c) -> p g c", c=18 * KGRP)
                n_grp = q // KGRP
                for g in range(n_grp):
                    nc.tensor.matmul(
                        ps[:],
                        av[:, g, :],
                        g1hv[:, g, :],
                        start=(t == 0 and g == 0),
                        stop=(t == n_tiles - 1 and g == n_grp - 1),
                    )

            if mode in ("full", "novec"):
                nc.scalar.copy(stage[:, b * FD : (b + 1) * FD], ps[:])

        nc.sync.dma_start(out[:], stage[:])

    nc.compile()
    return nc


_NC_CACHE = {}


def _get_nc():
    if "nc" not in _NC_CACHE:
        _NC_CACHE["nc"] = build_nc()
    return _NC_CACHE["nc"]


def fixup(Pm):
    """[Bt,FD,FD] grouped outer products -> [Bt,9,2] complex C entries.

    The KGRP diagonal [18,18] blocks each hold a partial sum over voxels of
    P18[b, 2*(3i+j)+ta, 2*(3j'+k)+tb] = sum_v A_ta[v,i,j] * Grf_tb[v,j',k].
    """
    Bt = Pm.shape[0]
    P18 = np.zeros((Bt, 18, 18), Pm.dtype)
    for k in range(KGRP):
        P18 += Pm[:, 18 * k : 18 * k + 18, 18 * k : 18 * k + 18]
    ii, kk = np.mgrid[0:3, 0:3]
    cr = np.zeros((Bt, 3, 3), np.float32)
    ci = np.zeros((Bt, 3, 3), np.float32)
    for j in range(3):
        ae = 6 * ii + 2 * j
        be = 6 * j + 2 * kk
        cr += P18[:, ae, be] - P18[:, ae + 1, be + 1]
        ci += P18[:, ae + 1, be] + P18[:, ae, be + 1]
    return np.stack([cr.reshape(Bt, 9), ci.reshape(Bt, 9)], axis=-1)


def run(x0, x1, x2, trace=False):
    from concourse.bass_utils import run_bass_kernel_spmd

    x0 = np.ascontiguousarray(np.asarray(x0), dtype=np.float32)
    x1 = np.ascontiguousarray(np.asarray(x1), dtype=np.float32)
    x2 = np.ascontiguousarray(np.asarray(x2), dtype=np.float32)
    assert x0.shape == (B_FULL, N_FULL, 9, 2), x0.shape

    nc = _get_nc()
    in_maps = [
        {
            "x0": x0[i * B_PC : (i + 1) * B_PC],
            "x1": x1[i * B_PC : (i + 1) * B_PC],
            "x2": x2[i * B_PC : (i + 1) * B_PC],
        }
        for i in range(N_CORES)
    ]
    res = run_bass_kernel_spmd(
        nc, in_maps, core_ids=list(range(N_CORES)), trace=trace
    )
    FD = 18 * KGRP
    Pm = np.concatenate(
        [
            res.results[i]["out"].reshape(FD, B_PC, FD).transpose(1, 0, 2)
            for i in range(N_CORES)
        ],
        axis=0,
    )
    return fixup(Pm), res


def kernel(x0, x1, x2):
    out, _ = run(x0, x1, x2, trace=False)
    return out


def _make_sharded_fn(nc, n_cores=N_CORES, donate=False, repeat=1):
    """Mirror bass2jax.run_bass_via_pjrt's multi-core lowering, returning a
    reusable jitted callable plus metadata, so we can time repeated runs on
    persistent device buffers."""
    import jax
    import jax.core
    from jax.experimental.shard_map import shard_map
    from jax.sharding import Mesh, PartitionSpec

    from concourse import bass2jax, mybir

    bass2jax.install_neuronx_cc_hook()

    partition_name = (
        nc.partition_id_tensor.name if nc.partition_id_tensor else None
    )
    in_names, out_names, out_avals, zero_outs = [], [], [], []
    for alloc in nc.m.functions[0].allocations:
        if not isinstance(alloc, mybir.MemoryLocationSet):
            continue
        name = alloc.memorylocations[0].name
        if alloc.kind == "ExternalInput":
            if name != partition_name:
                in_names.append(name)
        elif alloc.kind == "ExternalOutput":
            shape = tuple(alloc.tensor_shape)
            dtype = mybir.dt.np(alloc.dtype)
            out_names.append(name)
            out_avals.append(jax.core.ShapedArray(shape, dtype))
            zero_outs.append(np.zeros(shape, dtype))
    n_params = len(in_names)
    all_in_names = list(in_names) + list(out_names)
    if partition_name is not None:
        all_in_names.append(partition_name)

    def _body(*args):
        ins = list(args[:n_params])
        prev_outs = list(args[n_params:])
        # `repeat` chained executions of the same NEFF inside one XLA
        # program: each round's outputs feed the next round's (donated-zero)
        # output operands, which defeats CSE and serializes the rounds, so
        # wall-time slope over `repeat` isolates pure on-device time.
        for _ in range(repeat):
            operands = ins + prev_outs
            if partition_name is not None:
                operands.append(bass2jax.partition_id_tensor())
            prev_outs = list(
                bass2jax._bass_exec_p.bind(
                    *operands,
                    out_avals=tuple(out_avals),
                    in_names=tuple(all_in_names),
                    out_names=tuple(out_names),
                    lowering_input_output_aliases=(),
                    sim_require_finite=True,
                    sim_require_nnan=True,
                    nc=nc,
                )
            )
        return tuple(prev_outs)

    devices = jax.devices()[:n_cores]
    mesh = Mesh(np.asarray(devices), ("core",))
    in_specs = (PartitionSpec("core"),) * (n_params + len(out_names))
    out_specs = (PartitionSpec("core"),) * len(out_names)
    donate_argnums = (
        tuple(range(n_params, n_params + len(out_names))) if donate else ()
    )
    fn = jax.jit(
        shard_map(
            _body, mesh=mesh, in_specs=in_specs, out_specs=out_specs,
            check_rep=False,
        ),
        donate_argnums=donate_argnums,
        keep_unused=True,
    )
    return fn, mesh, in_names, out_names, zero_outs


def bench(x0, x1, x2, repeats=(1, 16), calls=20, nc=None):
    """Time the NEFF on-device via the repeat-slope method.

    Builds two XLA programs that chain R executions of the same NEFF
    back-to-back on device; per-call dispatch overhead is identical for
    both, so exec_ns = (T(R2) - T(R1)) / (R2 - R1) is pure HW time.
    """
    import time

    import jax
    from jax.sharding import NamedSharding, PartitionSpec

    x0 = np.ascontiguousarray(np.asarray(x0), dtype=np.float32)
    x1 = np.ascontiguousarray(np.asarray(x1), dtype=np.float32)
    x2 = np.ascontiguousarray(np.asarray(x2), dtype=np.float32)
    if nc is None:
        nc = _get_nc()
    concat = {"x0": x0, "x1": x1, "x2": x2}

    per_call = {}
    out = None
    for R in repeats:
        nc_r = nc if R == 1 else build_nc(repeat=R)
        fn, mesh, in_names, out_names, zero_outs = _make_sharded_fn(nc_r)
        sh = NamedSharding(mesh, PartitionSpec("core"))
        args = [jax.device_put(concat[n], sh) for n in in_names]
        args += [
            jax.device_put(
                np.zeros((N_CORES * z.shape[0], *z.shape[1:]), z.dtype), sh
            )
            for z in zero_outs
        ]
        out = fn(*args)
        jax.block_until_ready(out)  # compile + warm
        best = float("inf")
        for _ in range(3):
            t0 = time.perf_counter()
            for _ in range(calls):
                out = fn(*args)
            jax.block_until_ready(out)
            best = min(best, (time.perf_counter() - t0) / calls)
        per_call[R] = best

    rs = sorted(per_call)
    per_exec = (per_call[rs[-1]] - per_call[rs[0]]) / (rs[-1] - rs[0])
    return per_exec * 1e9, {r: f"{v*1e6:.0f}us" for r, v in per_call.items()}, (
        np.asarray(out[0]) if out is not None else None
    )

